# revision 3
# baseline (speedup 1.0000x reference)
"""Fused biased-softmax attention (nn_Attention_55576876810478) on 8 TRN2 NeuronCores.

Tensor-parallel by head (H=8 -> 1 head/core).  v2 rewrite of the baseline:

  * No SBUF->SBUF gpsimd remaps: every tensor is produced directly in the
    layout its consumer needs.
      - k projection uses a [0 | wk] zero-padded stationary so kT lands on
        partitions 32:64, matching qT (from the [wg | wq] fused stationary),
        so score matmuls read both operands at base partition 32.
      - v is projected TRANSPOSED (stationary = kvxT chunk, moving = wv),
        landing [k%128, c] tiles directly in the PV stationary layout.
  * Scores PSUM tile is [128, 2048] (4 k-tiles, 4 banks, single buffer):
    one ACT exp per 4 k-tiles, one DVE bias-multiply per 4 k-tiles.
  * Softmax denominators ride row 32 of the PV accumulator (ones-column
    trick) into odn; normalization happens ON THE HOST (out rows divided by
    den before the partial sums), removing the den-transpose/reciprocal/
    per-tile scale chain from the device entirely.
  * Output written unscaled bf16, 2 row-tiles per DVE evac, one contiguous
    256KB DMA per 512 q rows.
  * Input/bias DMAs are issued in deadline order (x chunks for batch b, then
    bias tiles for batch b).
"""

import math

import ml_dtypes
import numpy as np

B, Q, KL, D, H, C = 4, 1024, 1024, 256, 8, 32
NCORES = 8
BQ = B * Q
BK = B * KL
NCH = 8            # 512-wide chunks of BQ / BK
CH = 512

_BF16 = ml_dtypes.bfloat16
_CACHE = {}


def _build_nc():
    import concourse.bass as bass  # noqa: F401
    import concourse.mybir as mybir
    import concourse.tile as tile
    from concourse.bacc import Bacc

    bf16 = mybir.dt.bfloat16
    f32 = mybir.dt.float32
    AF = mybir.ActivationFunctionType
    ALU = mybir.AluOpType

    nc = Bacc(None, target_bir_lowering=False)

    qxT_d = nc.dram_tensor("qxT", [NCH, 128, 2, CH], bf16, kind="ExternalInput")
    kvxT_d = nc.dram_tensor("kvxT", [NCH, 128, 2, CH], bf16, kind="ExternalInput")
    # bias tile (b, jq, pkk): [k%128, 4*512] covering k-tiles 4pkk..4pkk+3 of
    # batch b, q window 512*jq..+512, pre-exp'd: exp(bias_pair + bias_mask)
    bias_d = nc.dram_tensor("bias", [B, 2, 2, 128, 2048], bf16,
                            kind="ExternalInput")
    # all weights in one DMA: cols [0:128]=wA(dc), [128:256]=wB(dc),
    # [256:320]=wv(dc), [320:576]=wo (rows 0:32), [576:578]=bg f32 (rows 0:32)
    wpk_d = nc.dram_tensor("wpk", [128, 578], bf16, kind="ExternalInput")
    out_d = nc.dram_tensor("out", [NCH, 128, 4, D], bf16, kind="ExternalOutput")
    den_d = nc.dram_tensor("den", [1, BQ], bf16, kind="ExternalOutput")

    with tile.TileContext(nc) as tc:
        with (
            tc.tile_pool(name="const", bufs=1) as const,
            tc.tile_pool(name="proj", bufs=1) as proj,
            tc.tile_pool(name="biasp", bufs=16) as biasp,
            tc.tile_pool(name="pp", bufs=3) as pp,
            tc.tile_pool(name="ppp", bufs=4) as ppp,
            tc.tile_pool(name="outp", bufs=2) as outp,
            tc.tile_pool(name="ps_s", bufs=2, space="PSUM") as ps_s,
            tc.tile_pool(name="ps_pv", bufs=2, space="PSUM") as ps_pv,
            tc.tile_pool(name="ps_pf", bufs=2, space="PSUM") as ps_pf,
        ):
            # ------------- persistent SBUF -------------
            qT = proj.tile([64, BQ], bf16)       # rows 32:64 = q / sqrt(C)
            kT = proj.tile([64, BK], bf16)       # rows 32:64 = k
            gT = proj.tile([33, BQ], bf16)       # gate rows 0:32, row 32 = 1
            vones = proj.tile([128, 32, 33], bf16)  # [k%128, ktile, c|ones]
            odn = proj.tile([33, BQ], bf16)      # gated O^T + den in row 32
            qxT = const.tile([128, NCH, 2, CH], bf16)
            kvxT = const.tile([128, NCH, 2, CH], bf16)
            wpk = const.tile([128, 578], bf16)
            wA = wpk[:, 0:128].rearrange("p (dc m) -> p dc m", dc=2)
            wB = wpk[:, 128:256].rearrange("p (dc m) -> p dc m", dc=2)
            wv = wpk[:, 256:320].rearrange("p (dc m) -> p dc m", dc=2)
            wo = wpk[0:C, 320:576]
            bg = wpk[0:C, 576:578].bitcast(f32)

            nc.gpsimd.memset(gT[32:33, :], 1.0)
            nc.vector.memset(vones[:, :, 32:33], 1.0)

            # ------------- DMA issue (deadline order) -------------
            nc.sync.dma_start(wpk, wpk_d[:, :])
            bias_tiles = {}
            for b in range(B):
                for j in (2 * b, 2 * b + 1):
                    nc.sync.dma_start(kvxT[:, j], kvxT_d[j])
                    nc.sync.dma_start(qxT[:, j], qxT_d[j])
                for jq in range(2):
                    for pkk in range(2):
                        bt = biasp.tile([128, 2048], bf16, tag="bias",
                                        name=f"bias_{b}_{jq}_{pkk}")
                        nc.sync.dma_start(bt, bias_d[b, jq, pkk])
                        bias_tiles[(b, jq, pkk)] = bt

            # ------------- projection chunk -------------
            def emit_proj(j):
                jsl = slice(j * CH, (j + 1) * CH)
                B_ps = ps_pf.tile([64, CH], f32, tag="pf", name=f"bps_{j}")
                for dc in range(2):
                    nc.tensor.matmul(B_ps, wB[:, dc, :], kvxT[:, j, dc, :],
                                     start=dc == 0, stop=dc == 1)
                nc.vector.tensor_copy(kT[32:64, jsl], B_ps[32:64, :])
                A_ps = ps_pf.tile([64, CH], f32, tag="pf", name=f"aps_{j}")
                for dc in range(2):
                    nc.tensor.matmul(A_ps, wA[:, dc, :], qxT[:, j, dc, :],
                                     start=dc == 0, stop=dc == 1)
                # sigmoid(x) = 0.5*tanh(0.5x) + 0.5 (tanh shares the exp
                # ACT table set)
                nc.scalar.activation(gT[0:32, jsl], A_ps[0:32, :], AF.Tanh,
                                     bias=bg, scale=0.5)
                nc.gpsimd.tensor_scalar(gT[0:32, jsl], gT[0:32, jsl], 0.5, 0.5,
                                        op0=ALU.mult, op1=ALU.add)
                nc.vector.tensor_copy(qT[32:64, jsl], A_ps[32:64, :])
                V_ps = ps_pf.tile([128, 128], f32, tag="pf", name=f"vps_{j}")
                for g4 in range(4):
                    for dc in range(2):
                        nc.tensor.matmul(
                            V_ps[:, 32 * g4:32 * (g4 + 1)],
                            kvxT[:, j, dc, 128 * g4:128 * (g4 + 1)],
                            wv[:, dc, :], start=dc == 0, stop=dc == 1)
                vv = V_ps.rearrange("p (g c) -> p g c", g=4)
                nc.vector.tensor_copy(vones[:, 4 * j:4 * j + 4, 0:C], vv)

            # ------------- attention window (b, jq) -------------
            # Two deferred-work queues so the PE never waits on the
            # exp->mult chain: part1(w) = pv accumulation of the second
            # k-half + the gated PV eviction; part2(w) = output projection.
            part1q = []
            part2q = []

            def flush(q):
                while q:
                    q.pop(0)()

            def emit_window(b, jq):
                t = 2 * b + jq
                qsl = slice(b * Q + jq * CH, b * Q + (jq + 1) * CH)
                pv = ps_pv.tile([33, CH], f32, tag="pv", name=f"pv_{b}_{jq}")
                ps = {}

                def half(pkk, h, mul_eng):
                    # 2 score matmuls -> exp -> bias-multiply for k-tiles
                    # (4pkk+2h, 4pkk+2h+1); fine granularity keeps the
                    # scores->p chain latency under the PE's cover work
                    s = ps_s.tile([128, 1024], f32, tag="s",
                                  name=f"s_{b}_{jq}_{pkk}_{h}")
                    for v in (2 * h, 2 * h + 1):
                        kt = 4 * pkk + v
                        nc.tensor.matmul(
                            s[:, 512 * (v - 2 * h):512 * (v - 2 * h + 1)],
                            kT[32:64, b * KL + 128 * kt:b * KL + 128 * (kt + 1)],
                            qT[32:64, qsl], start=True, stop=True)
                    praw = pp.tile([128, 1024], bf16, tag="praw",
                                   name=f"praw_{b}_{jq}_{pkk}_{h}")
                    nc.scalar.activation(praw, s, AF.Exp)
                    p = ppp.tile([128, 1024], bf16, tag="p",
                                 name=f"p_{b}_{jq}_{pkk}_{h}")
                    bt = bias_tiles[(b, jq, pkk)]
                    mul_eng.tensor_mul(p, praw, bt[:, 1024 * h:1024 * (h + 1)])
                    ps[(pkk, h)] = p

                def pvmm(pkk, h, first, last):
                    for v in (0, 1):
                        nc.tensor.matmul(
                            pv, vones[:, 8 * b + 4 * pkk + 2 * h + v, :],
                            ps[(pkk, h)][:, 512 * v:512 * (v + 1)],
                            start=first and v == 0,
                            stop=last and v == 1)

                half(0, 0, nc.vector)
                half(0, 1, nc.vector)
                flush(part1q)  # prev window: pv second half + odn
                half(1, 0, nc.vector)
                pvmm(0, 0, True, False)
                half(1, 1, nc.gpsimd)
                pvmm(0, 1, False, False)
                flush(part2q)  # prev window: out-projection

                def part1():
                    pvmm(1, 0, False, False)
                    pvmm(1, 1, False, True)
                    nc.vector.scalar_tensor_tensor(
                        odn[:, qsl], pv, 1.0, gT[:, qsl],
                        op0=ALU.mult, op1=ALU.mult)

                def part2():
                    ots = outp.tile([128, 4, D], bf16, tag="ot",
                                    name=f"ots_{t}")
                    og = odn[0:C, qsl].rearrange("c (p j) -> c j p", j=4)
                    for fp in range(2):  # pairs of row-tiles
                        fo = ps_pf.tile([128, 2 * D], f32, tag="pf",
                                        name=f"fo_{t}_{fp}")
                        for i in range(2):
                            nc.tensor.matmul(fo[:, D * i:D * (i + 1)],
                                             og[:, 2 * fp + i, :], wo,
                                             start=True, stop=True)
                        nc.vector.tensor_copy(
                            ots[:, 2 * fp:2 * fp + 2, :],
                            fo.rearrange("p (i d) -> p i d", i=2))
                    # SWDGE: keeps output writes off the HWDGE queues, which
                    # are backed up with input descriptors until ~t+47us
                    nc.gpsimd.dma_start(out_d[t], ots)

                part1q.append(part1)
                part2q.append(part2)

            # ------------- emission order -------------
            emit_proj(0)
            emit_proj(1)
            # chunks 2,3 fill window (0,0)'s empty flush slots
            part1q.append(lambda: emit_proj(2))
            part2q.append(lambda: emit_proj(3))
            for b in range(B):
                if b >= 2:
                    emit_proj(2 * b)
                    emit_proj(2 * b + 1)
                for jq in range(2):
                    emit_window(b, jq)
            flush(part1q)
            flush(part2q)
            nc.gpsimd.dma_start(den_d[:, :], odn[32:33, :])

    nc.finalize()
    return nc


def _get_nc():
    if "nc" not in _CACHE:
        _CACHE["nc"] = _build_nc()
    return _CACHE["nc"]


def _prep(inputs):
    q_x = np.asarray(inputs["q_x"], np.float32)
    kv_x = np.asarray(inputs["kv_x"], np.float32)
    bias_mask = np.asarray(inputs["bias_mask"], np.float32)
    bias_pair = np.asarray(inputs["bias_pair"], np.float32)
    wq = np.asarray(inputs["wq"], np.float32)
    wk = np.asarray(inputs["wk"], np.float32)
    wv = np.asarray(inputs["wv"], np.float32)
    wg = np.asarray(inputs["wg"], np.float32)
    bg = np.asarray(inputs["bg"], np.float32)
    wo = np.asarray(inputs["wo"], np.float32)
    sc = 1.0 / math.sqrt(C)

    # x chunks: [j, p, dc, t] = x[512j + t, 128dc + p]
    def chunkify(x):
        return np.ascontiguousarray(
            x.reshape(BQ, D).reshape(NCH, CH, 2, 128).transpose(0, 3, 2, 1)
        ).astype(_BF16)

    qxT = chunkify(q_x)
    kvxT = chunkify(kv_x)
    bmk = bias_mask.reshape(B, KL)

    in_maps = []
    for h in range(NCORES):
        csl = slice(h * C, (h + 1) * C)
        # bias [b, jq, pkk, kp, 512u + q'] = e[k=128(4pkk+u)+kp, q=512jq+q']
        bias = np.empty((B, 2, 2, 128, 2048), np.float32)
        for b in range(B):
            e = np.exp(bias_pair[b, h] + bmk[b][None, :])  # [Q, K]
            eT = e.T.reshape(2, 4, 128, 2, 512)  # [pkk, u, kp, jq, q']
            bias[b] = eT.transpose(3, 0, 2, 1, 4).reshape(2, 2, 128, 2048)
        wA = np.concatenate([wg[:, csl], wq[:, csl] * sc], axis=1)
        wB = np.concatenate([np.zeros((D, C), np.float32), wk[:, csl]], axis=1)
        wpk = np.zeros((128, 578), _BF16)
        # [p, 64*dc + m] = w[128*dc + p, m]
        wpk[:, 0:128] = wA.reshape(2, 128, 64).transpose(1, 0, 2).reshape(
            128, 128).astype(_BF16)
        wpk[:, 128:256] = wB.reshape(2, 128, 64).transpose(1, 0, 2).reshape(
            128, 128).astype(_BF16)
        wpk[:, 256:320] = wv[:, csl].reshape(2, 128, C).transpose(
            1, 0, 2).reshape(128, 64).astype(_BF16)
        wpk[0:C, 320:576] = wo[csl, :].astype(_BF16)
        wpk[0:C, 576:578] = (0.5 * bg[csl].astype(np.float32)).reshape(
            C, 1).view(np.uint16).view(_BF16)
        in_maps.append({
            "qxT": qxT,
            "kvxT": kvxT,
            "bias": bias.astype(_BF16),
            "wpk": wpk,
        })
    return in_maps


def _run(inputs, trace=False, **kw):
    from concourse.bass_utils import run_bass_kernel_spmd

    in_maps = _prep(inputs)
    nc = _get_nc()
    r = run_bass_kernel_spmd(nc, in_maps, core_ids=list(range(NCORES)),
                             trace=trace, **kw)
    bo = np.asarray(inputs["bo"], np.float32)
    total = np.zeros((BQ, D), np.float32)
    for i in range(NCORES):
        part = r.results[i]["out"].reshape(BQ, D).astype(np.float32)
        den = r.results[i]["den"].reshape(BQ, 1).astype(np.float32)
        total += part / den
    total += bo
    return total.reshape(B, Q, D).astype(np.float32), r


def kernel(**inputs):
    out, _ = _run(inputs, trace=False)
    return out
